# revision 12
# baseline (speedup 1.0000x reference)
"""Bi-directional multi-head cross-attention (GLIP-style) on 8 Trainium2 cores.

Shapes (hardcoded): B=2, TV=16000, TL=256, EMBED=256, H=8, D=32, fp32.

Sharding: data-parallel over batch (2) x vision-sequence-parallel (4) = 8
cores; each core takes a 4096-row slice of the zero-padded (16384) vision
tokens of one batch. The vision-direction softmax (over TL) is core-local;
the lang-direction softmax reduces over the sharded TV axis, so each core
emits partial numerator N[s,h,d] = sum_t E*vv and denominator S[s,h] =
sum_t E (E = exp(score)); the host sums these small partials (270KB/core)
across the 4 cores of each batch during the gather and applies the tiny
256x256 lang output projection.

Scores use a folded matrix A = Wq_h @ k_h^T (per head, built on-device once
per launch) so the per-tile score matmuls contract the full 256-wide
embedding with no sub-partition operands. Padding rows get an additive
-30000 score bias fused into the exp (per-partition ACT bias), zeroing
their lang-direction contribution exactly; their out_v rows are trimmed by
the host. Nonzero projection biases / real attention masks never occur for
this problem's generated inputs; the host falls back to a numpy reference
if they appear.
"""

import sys
import numpy as np

sys.path.insert(0, "/opt/trn_rl_repo")

import concourse.bass as bass  # noqa: E402
import concourse.mybir as mybir  # noqa: E402
import concourse.tile as tile  # noqa: E402
from concourse import bacc  # noqa: E402
from concourse.masks import make_identity  # noqa: E402
from concourse.bass_utils import run_bass_kernel_spmd  # noqa: E402

B, TV, TL, E, H, D = 2, 16000, 256, 256, 8, 32
T_PAD = 16384
T_C = T_PAD // 4          # 4096 vision tokens per core
N_CHUNK = T_C // 128      # 32 chunks of 128 tokens
N_G512 = T_C // 512       # 8 groups of 512 tokens
SCALE = float(D) ** -0.5
NEG_BIAS = -30000.0

F32 = mybir.dt.float32
F32R = mybir.dt.float32r
BF16 = mybir.dt.bfloat16

AluOp = mybir.AluOpType
ActFn = mybir.ActivationFunctionType

_CACHE = {}


def _build_nc():
    import contextlib

    nc = bacc.Bacc("TRN2", target_bir_lowering=False, debug=False, num_devices=8)

    visionT = nc.dram_tensor("visionT", [2, 128, T_C], F32, kind="ExternalInput")
    langT = nc.dram_tensor("langT", [2, 128, TL], F32, kind="ExternalInput")
    wq = nc.dram_tensor("wq", [2, 128, E], F32, kind="ExternalInput")
    wk = nc.dram_tensor("wk", [2, 128, E], F32, kind="ExternalInput")
    wvv = nc.dram_tensor("wvv", [2, 128, E], F32, kind="ExternalInput")
    wvl = nc.dram_tensor("wvl", [2, 128, E], F32, kind="ExternalInput")
    wov = nc.dram_tensor("wov", [2, 128, E], F32, kind="ExternalInput")
    keep_v = nc.dram_tensor("keep_v", [128, N_CHUNK], F32, kind="ExternalInput")

    out_v = nc.dram_tensor("out_v", [T_C, E], F32, kind="ExternalOutput")
    # lang partials: [s_half, s_in_half, 33h + (d | S)]
    nl_out = nc.dram_tensor("nl_out", [2, 128, 33 * H], F32, kind="ExternalOutput")

    ctx = contextlib.ExitStack()
    with tile.TileContext(nc) as tc, ctx:
        const = ctx.enter_context(tc.tile_pool(name="const", bufs=1))
        sb_in = ctx.enter_context(tc.tile_pool(name="sb_in", bufs=3))
        sb_e = ctx.enter_context(tc.tile_pool(name="sb_e", bufs=3))
        sb_est = ctx.enter_context(tc.tile_pool(name="sb_est", bufs=3))
        sb_wk = ctx.enter_context(tc.tile_pool(name="sb_wk", bufs=3))
        sb_out = ctx.enter_context(tc.tile_pool(name="sb_out", bufs=3))
        # PSUM (8 banks): num 2 | sc 2 | tr 1 | od 1 | work 2
        ps_num = ctx.enter_context(tc.tile_pool(name="ps_num", bufs=1, space="PSUM"))
        ps_sc = ctx.enter_context(tc.tile_pool(name="ps_sc", bufs=1, space="PSUM"))
        ps_tr = ctx.enter_context(tc.tile_pool(name="ps_tr", bufs=1, space="PSUM"))
        ps_od = ctx.enter_context(tc.tile_pool(name="ps_od", bufs=1, space="PSUM"))
        ps_wk = ctx.enter_context(tc.tile_pool(name="ps_wk", bufs=2, space="PSUM"))

        # ---------------- static prep ----------------
        identb = const.tile([128, 128], BF16)
        make_identity(nc, identb)
        identr0 = sb_wk.tile([128, 128], F32, tag="identr0")
        make_identity(nc, identr0)
        identr = const.tile([128, 128], F32R)
        nc.vector.tensor_copy(identr, identr0)
        zscratch = const.tile([128, 4096], F32)
        nc.vector.memset(zscratch, 0.0)
        onescratch = const.tile([128, 1024], F32)
        nc.vector.memset(onescratch, 1.0)
        ones_bf = const.tile([128, 33 * H], BF16)
        nc.vector.tensor_copy(ones_bf, onescratch[:, 0:33 * H])

        w_sb = {}
        for name, t in (("wq", wq), ("wk", wk), ("wvv", wvv), ("wvl", wvl), ("wov", wov)):
            raw = sb_in.tile([128, 2, E], F32, tag="w_raw")
            nc.sync.dma_start(out=raw, in_=t[:, :, :].rearrange("c p e -> p c e"))
            s = const.tile([128, 2, E], F32R, tag=f"w_{name}")
            nc.vector.tensor_copy(s.rearrange("p a b -> p (a b)"),
                                  raw.rearrange("p a b -> p (a b)"))
            w_sb[name] = s
        langT_raw = sb_in.tile([128, 2, TL], F32, tag="w_raw")
        nc.sync.dma_start(out=langT_raw, in_=langT[:, :, :].rearrange("c p e -> p c e"))
        langT_sb = const.tile([128, 2, TL], F32R)
        nc.vector.tensor_copy(langT_sb.rearrange("p a b -> p (a b)"),
                              langT_raw.rearrange("p a b -> p (a b)"))
        keep_raw = const.tile([128, N_CHUNK], F32)
        nc.sync.dma_start(out=keep_raw, in_=keep_v[:, :])
        keep_bf = const.tile([128, N_CHUNK], BF16)
        nc.vector.tensor_copy(keep_bf, keep_raw)

        # kT[e_q, s] = (Wk^T @ langT) * scale   (scale folded into k)
        kT_sb = const.tile([128, 2, TL], F32R)
        for j in range(2):
            p = ps_wk.tile([128, 512], F32, tag="work")
            for ei in range(2):
                nc.tensor.matmul(
                    p[:, 0:TL], w_sb["wk"][:, ei, 128 * j:128 * (j + 1)],
                    langT_sb[:, ei, :], start=(ei == 0), stop=(ei == 1),
                )
            nc.vector.tensor_scalar(
                out=kT_sb[:, j, :], in0=p[:, 0:TL], scalar1=SCALE, scalar2=None,
                op0=AluOp.mult,
            )

        # KD[e_q, (h, s)]: block-diagonal per-head copy of kT
        KD = const.tile([128, 2, H * TL], F32R)
        nc.vector.tensor_copy(KD.rearrange("p a b -> p (a b)"), zscratch)
        for h in range(H):
            nc.vector.tensor_copy(
                KD[32 * (h % 4):32 * (h % 4 + 1), h // 4, TL * h:TL * (h + 1)],
                kT_sb[32 * (h % 4):32 * (h % 4 + 1), h // 4, :],
            )

        # wqT via PE transpose (f32r)
        wqT_sb = const.tile([128, 2, E], F32R)
        for eco in range(2):
            pt = ps_wk.tile([128, 512], F32R, tag="work")
            for eci in range(2):
                nc.tensor.transpose(
                    pt[:, 128 * eci:128 * (eci + 1)],
                    w_sb["wq"][:, eci, 128 * eco:128 * (eco + 1)], identr,
                )
            nc.vector.tensor_copy(wqT_sb[:, eco, :], pt[:, 0:E])

        # A[e_in, (h, s)] = Wq @ KD  (score-folded matrix)
        A_sb = const.tile([128, 2, H * TL], F32R)
        for eb in range(2):
            for ns in range(4):
                pa = ps_wk.tile([128, 512], F32, tag="work")
                for qc in range(2):
                    nc.tensor.matmul(
                        pa, wqT_sb[:, qc, 128 * eb:128 * (eb + 1)],
                        KD[:, qc, 512 * ns:512 * (ns + 1)],
                        start=(qc == 0), stop=(qc == 1),
                    )
                nc.vector.tensor_copy(A_sb[:, eb, 512 * ns:512 * (ns + 1)], pa)

        # vl_ext[s_half, g, (h, d|ones)]: vl columns interleaved with ones
        # columns: per head 64 cols = [vl_h (32) | ones (32)]
        vl_sb = const.tile([128, 2, H * 64], BF16)
        nc.vector.tensor_copy(vl_sb.rearrange("p a b -> p (a b)"), onescratch)
        for g in range(2):
            p = ps_wk.tile([128, 512], F32, tag="work")
            for ei in range(2):
                nc.tensor.matmul(
                    p[:, 0:E], langT_sb[:, ei, 128 * g:128 * (g + 1)],
                    w_sb["wvl"][:, ei, :], start=(ei == 0), stop=(ei == 1),
                )
            nc.vector.tensor_copy(
                vl_sb[:, g, :].rearrange("p (h x) -> p h x", h=H)[:, :, 0:32],
                p[:, 0:E].rearrange("p (h d) -> p h d", h=H),
            )

        # standing accumulators: psum_N[g][s_in_half, 33h + (d|S)]
        psum_N = [ps_num.tile([128, 33 * H], F32, tag=f"num{g}", name=f"psum_N{g}")
                  for g in range(2)]

        # ---------------- main loop ----------------
        for g512 in range(N_G512):
            vT_raw = sb_in.tile([128, 2, 512], F32, tag="vT_raw")
            nc.sync.dma_start(
                out=vT_raw,
                in_=visionT[:, :, 512 * g512:512 * (g512 + 1)].rearrange("c p t -> p c t"),
            )
            vT = sb_in.tile([128, 2, 512], F32R, tag="vT")
            nc.vector.tensor_copy(vT.rearrange("p a b -> p (a b)"),
                                  vT_raw.rearrange("p a b -> p (a b)"))

            for sub in range(4):
                ci = 4 * g512 + sub
                tsl = slice(128 * sub, 128 * (sub + 1))

                # vv_ext [t, 33h + (d|1)] bf16 (ones col for S accumulation)
                pvv = ps_wk.tile([128, 512], F32, tag="work")
                for ei in range(2):
                    nc.tensor.matmul(
                        pvv[:, 0:E], vT[:, ei, tsl], w_sb["wvv"][:, ei, :],
                        start=(ei == 0), stop=(ei == 1),
                    )
                vvx = sb_wk.tile([128, 33 * H], BF16, tag="vvx")
                nc.vector.tensor_copy(vvx, ones_bf)
                keep_src = bass.AP(
                    tensor=keep_bf.tensor, offset=keep_bf.offset + ci,
                    ap=[keep_bf.ap[0], [0, H], [1, 1]],
                )
                nc.vector.tensor_copy(
                    vvx.rearrange("p (h x) -> p h x", h=H)[:, :, 32:33], keep_src)
                nc.vector.tensor_copy(
                    vvx.rearrange("p (h x) -> p h x", h=H)[:, :, 0:32],
                    pvv[:, 0:E].rearrange("p (h d) -> p h d", h=H),
                )

                norm_sb = sb_wk.tile([128, E], F32R, tag="norm")

                for rnd in range(2):  # heads 4*rnd .. 4*rnd+3
                    # scores [t=128, (4 heads x 256)]
                    sc = ps_sc.tile([128, 4 * TL], F32, tag="sc")
                    for ns in range(2):
                        for ei in range(2):
                            nc.tensor.matmul(
                                sc[:, 512 * ns:512 * (ns + 1)],
                                vT[:, ei, tsl],
                                A_sb[:, ei, 1024 * rnd + 512 * ns:1024 * rnd + 512 * (ns + 1)],
                                start=(ei == 0), stop=(ei == 1),
                            )
                    e_ts = sb_e.tile([128, 4 * TL], BF16, tag="e_ts")
                    nc.scalar.activation(out=e_ts, in_=sc, func=ActFn.Exp)

                    # lang numerator + S: lhsT = E-slice, rhs = vv_ext cols
                    for hj in range(4):
                        h = 4 * rnd + hj
                        for g in range(2):
                            nc.tensor.matmul(
                                psum_N[g][:, 33 * h:33 * (h + 1)],
                                e_ts[:, 256 * hj + 128 * g:256 * hj + 128 * (g + 1)],
                                vvx[:, 33 * h:33 * (h + 1)],
                                # start clears the whole bank's has_written
                                # bits: only the group's very first matmul
                                # may set it, else earlier regions' state is
                                # wiped and their next write overwrites
                                # instead of accumulating
                                start=(ci == 0 and rnd == 0 and hj == 0),
                                stop=(ci == N_CHUNK - 1 and rnd == 1 and hj == 3),
                            )

                    # transpose E -> e_st [s_in_half, (hj, half), t] bf16
                    e_st = sb_est.tile([128, 8, 128], BF16, tag="e_st")
                    for q in range(2):
                        pt = ps_tr.tile([128, 512], BF16, tag="tr")
                        for r in range(4):
                            blk = 4 * q + r
                            nc.tensor.transpose(
                                pt[:, 128 * r:128 * (r + 1)],
                                e_ts[:, 128 * blk:128 * (blk + 1)], identb,
                            )
                        cp = nc.vector if q == 0 else nc.scalar
                        if q == 0:
                            nc.vector.tensor_copy(
                                e_st[:, 0:4, :].rearrange("p a b -> p (a b)"), pt)
                        else:
                            nc.scalar.activation(
                                out=e_st[:, 4:8, :].rearrange("p a b -> p (a b)"),
                                in_=pt, func=ActFn.Copy)

                    # out_v numerator + dup'd denominator:
                    # od[t, 64hj + (d|dup)] += E_st_hj_half^T @ [vl_h | ones]
                    od = ps_od.tile([128, 4 * 64], F32, tag="od")
                    for hj in range(4):
                        h = 4 * rnd + hj
                        for half in range(2):
                            nc.tensor.matmul(
                                od[:, 64 * hj:64 * (hj + 1)],
                                e_st[:, 2 * hj + half, :],
                                vl_sb[:, half, 64 * h:64 * (h + 1)],
                                start=(half == 0), stop=(half == 1),
                            )
                    # normalize: norm[t, (4rnd+hj, d)] = ovt / den
                    recip = sb_wk.tile([128, 4, 32], F32, tag="recip")
                    odv = od.rearrange("p (j x) -> p j x", j=4)
                    nc.vector.reciprocal(out=recip, in_=odv[:, :, 32:64])
                    nc.vector.tensor_tensor(
                        out=norm_sb[:, 128 * rnd:128 * (rnd + 1)].rearrange(
                            "p (j d) -> p j d", j=4),
                        in0=odv[:, :, 0:32],
                        in1=recip,
                        op=AluOp.mult,
                    )

                # transpose norm -> [e, t] then project with Wov
                ptn = ps_wk.tile([128, 512], F32R, tag="work")
                for g in range(2):
                    nc.tensor.transpose(
                        ptn[:, 128 * g:128 * (g + 1)],
                        norm_sb[:, 128 * g:128 * (g + 1)], identr,
                    )
                normT = sb_wk.tile([128, 2, 128], F32R, tag="normT")
                nc.vector.tensor_copy(normT.rearrange("p a b -> p (a b)"),
                                      ptn[:, 0:256])
                pj = ps_wk.tile([128, 512], F32, tag="work")
                for g in range(2):
                    nc.tensor.matmul(
                        pj[:, 0:E], normT[:, g, :], w_sb["wov"][:, g, :],
                        start=(g == 0), stop=(g == 1),
                    )
                o_sb = sb_out.tile([128, E], F32, tag="o_sb")
                nc.vector.tensor_copy(o_sb, pj[:, 0:E])
                nc.sync.dma_start(out=out_v[128 * ci:128 * (ci + 1), :], in_=o_sb)

        # ---- flush lang partials ----
        for g in range(2):
            nl_sb = sb_out.tile([128, 33 * H], F32, tag="nl")
            nc.vector.tensor_copy(nl_sb, psum_N[g])
            nc.sync.dma_start(out=nl_out[g, :, :], in_=nl_sb)

    nc.finalize()
    return nc


def _numpy_reference(vision, lang, attention_mask_v, attention_mask_l,
                     Wq, bq, Wk, bk, Wvv, bvv, Wvl, bvl, Wov, bov, Wol, bol):
    bsz, tv, _ = vision.shape
    tl = lang.shape[1]
    scale = D ** -0.5

    def heads(x, slen):
        return x.reshape(bsz, slen, H, D).transpose(0, 2, 1, 3)

    q = heads((vision @ Wq + bq) * scale, tv)
    k = heads(lang @ Wk + bk, tl)
    vv = heads(vision @ Wvv + bvv, tv)
    vl = heads(lang @ Wvl + bvl, tl)
    attn = np.einsum("bhtd,bhsd->bhts", q, k)
    attn = np.clip(attn, -50000.0, 50000.0)
    attn_t = attn.transpose(0, 1, 3, 2)
    attn_l = attn_t - attn_t.max(-1, keepdims=True)
    attn_l = np.clip(attn_l, -50000.0, 50000.0)
    attn_l = np.where(attention_mask_v[:, None, None, :], -np.inf, attn_l)
    attn_l = np.exp(attn_l - attn_l.max(-1, keepdims=True))
    attn_l = attn_l / attn_l.sum(-1, keepdims=True)
    mask_l = np.where(attention_mask_l == 0, -9e15, 0.0).astype(attn.dtype)
    av = attn + mask_l[:, None, None, :]
    av = np.exp(av - av.max(-1, keepdims=True))
    attn_v = av / av.sum(-1, keepdims=True)
    out_v = np.einsum("bhts,bhsd->bhtd", attn_v, vl)
    out_l = np.einsum("bhst,bhtd->bhsd", attn_l, vv)
    out_v = out_v.transpose(0, 2, 1, 3).reshape(bsz, tv, E) @ Wov + bov
    out_l = out_l.transpose(0, 2, 1, 3).reshape(bsz, tl, E) @ Wol + bol
    return out_v.astype(np.float32), out_l.astype(np.float32)


def kernel(vision, lang, attention_mask_v, attention_mask_l,
           Wq, bq, Wk, bk, Wvv, bvv, Wvl, bvl, Wov, bov, Wol, bol):
    vision = np.asarray(vision, dtype=np.float32)
    lang = np.asarray(lang, dtype=np.float32)
    attention_mask_v = np.asarray(attention_mask_v)
    attention_mask_l = np.asarray(attention_mask_l)
    args = [np.asarray(a, dtype=np.float32)
            for a in (Wq, bq, Wk, bk, Wvv, bvv, Wvl, bvl, Wov, bov, Wol, bol)]
    (Wq, bq, Wk, bk, Wvv, bvv, Wvl, bvl, Wov, bov, Wol, bol) = args

    # device kernel assumes no masking and zero in-projection biases (always
    # true for this problem's generated inputs)
    if (attention_mask_v.astype(bool).any() or (attention_mask_l == 0).any()
            or any(np.any(b) for b in (bq, bk, bvv, bvl))):
        return _numpy_reference(
            vision, lang, attention_mask_v, attention_mask_l,
            Wq, bq, Wk, bk, Wvv, bvv, Wvl, bvl, Wov, bov, Wol, bol)

    if "nc" not in _CACHE:
        _CACHE["nc"] = _build_nc()
    nc = _CACHE["nc"]

    shared = {
        "wq": Wq.reshape(2, 128, E), "wk": Wk.reshape(2, 128, E),
        "wvv": Wvv.reshape(2, 128, E), "wvl": Wvl.reshape(2, 128, E),
        "wov": Wov.reshape(2, 128, E),
    }
    vis_pad = np.zeros((B, T_PAD, E), np.float32)
    vis_pad[:, :TV, :] = vision
    keep_full = np.zeros(T_PAD, np.float32)
    keep_full[:TV] = 1.0
    in_maps = []
    for c in range(8):
        b, g = c // 4, c % 4
        t0 = g * T_C
        vT = np.ascontiguousarray(vis_pad[b, t0:t0 + T_C, :].T).reshape(2, 128, T_C)
        keep_c = np.ascontiguousarray(keep_full[t0:t0 + T_C].reshape(N_CHUNK, 128).T)
        in_maps.append({
            **shared,
            "visionT": vT,
            "langT": np.ascontiguousarray(lang[b].T).reshape(2, 128, TL),
            "keep_v": keep_c,
        })

    res = run_bass_kernel_spmd(nc, in_maps, list(range(8))).results

    out_v = np.empty((B, TV, E), np.float32)
    out_l = np.empty((B, TL, E), np.float32)
    for b in range(B):
        parts_v = [res[4 * b + g]["out_v"] for g in range(4)]
        out_v[b] = np.concatenate(parts_v, axis=0)[:TV]
        if bov.any():
            out_v[b] += bov
        NS = np.zeros((2, 128, 33 * H), np.float64)
        for g in range(4):
            NS += res[4 * b + g]["nl_out"]
        NS = NS.reshape(2 * 128, H, 33)           # [s, h, 33]
        N = NS[:, :, 0:32]                        # [s, h, d]
        S = NS[:, :, 32]                          # [s, h]
        attn_out = (N / S[:, :, None]).reshape(TL, E).astype(np.float32)
        out_l[b] = attn_out @ Wol + bol
    return out_v, out_l
